# revision 11
# baseline (speedup 1.0000x reference)
"""Causal multi-head self-attention with RoPE on 8 Trainium2 NeuronCores.

Problem: x[2, 2048, 1024] fp32, 16 heads, d_head=64, causal, RoPE(theta=1e4).
Sharding: core = b*4 + g  (b in {0,1} batch, g in {0..3} head-group of 4 heads).
Each core computes out_partial[2048, 1024] = attn(heads of g) @ wo[:, cols_g].T;
host sums the 4 partials per batch.

Per-core kernel (all matmuls in fp32r, 1 cycle/row):
  B) Q/K projections into [d_head, seq] layout (2 heads per 128 partitions)
     with RoPE fused:  q_rot = A*cosT + P@(A*sinT)  (P = pair-swap sign matrix,
     applied via a single PE matmul; tables are pair-symmetric so P commutes
     with the elementwise sin multiply).
  C) V projection into [seq_tile(128) partitions, 4*64+ones] layout; the
     ones column makes the second attention matmul also produce the softmax
     denominator for free.
  D) Per (head, q-chunk of 512): scores_T[k_tile 128, q 512] = K_tile @ Q_chunk
     via PE (contraction d=64), exp on ACT (scale=1/8 fused), causal mask by
     multiplying precomputed 0/1 masks on diagonal tiles, then
     attn_aug[65, 512] += V_aug.T @ probs_T accumulated in PSUM over k tiles.
     Normalize with DVE reciprocal + gpsimd partition broadcast.
  E) out_partial = attnT.T @ wo_t, tiled 128x512, accumulated over 2 k-subtiles.
"""

import os
import sys

sys.path.insert(0, "/opt/trn_rl_repo")

import numpy as np

import concourse.bass as bass
import concourse.bacc as bacc
import concourse.mybir as mybir
from concourse import library_config
from concourse.tile import TileContext

B = 2
S = 2048
DM = 1024
H = 16
DH = 64
HLOC = 4  # heads per core
SC = 512  # q chunk size
NKT = S // 128  # 16 k tiles
NQC = S // SC  # 4 q chunks
P = 128
KO = DM // P  # 8 contraction subtiles for projections
SCALE = 1.0 / 8.0  # 1/sqrt(DH)
THETA = 10000.0

F32 = mybir.dt.float32
F32R = mybir.dt.float32r

_CACHE = {}


def _build_nc():
    nc = bacc.Bacc("TRN2", enable_partition_id=False)
    Exp = mybir.ActivationFunctionType.Exp

    xT = nc.dram_tensor("xT", [DM, S], F32R, kind="ExternalInput")
    wq_t = nc.dram_tensor("wq_t", [DM, 256], F32R, kind="ExternalInput")
    wk_t = nc.dram_tensor("wk_t", [DM, 256], F32R, kind="ExternalInput")
    wv_t = nc.dram_tensor("wv_t", [DM, 256], F32R, kind="ExternalInput")
    wo_t = nc.dram_tensor("wo_t", [256, DM], F32R, kind="ExternalInput")
    cosT = nc.dram_tensor("cosT", [P, S], F32, kind="ExternalInput")
    sinT = nc.dram_tensor("sinT", [P, S], F32, kind="ExternalInput")
    perm = nc.dram_tensor("perm", [P, P], F32R, kind="ExternalInput")
    masks = nc.dram_tensor("masks", [NQC, P, SC], F32, kind="ExternalInput")
    outp = nc.dram_tensor("out_partial", [S, DM], F32, kind="ExternalOutput")

    with TileContext(nc) as tc:
        with tc.tile_pool(name="persist", bufs=1) as persist:
            # [pair-head-dim (2*64), head-pair, seq]
            q_rot = persist.tile([P, 2, S], F32R, tag="q_rot")
            k_rot = persist.tile([P, 2, S], F32R, tag="k_rot")
            # V in [k partitions, k_tile, head, 72]: cols 0:64 = V, 64 = ones
            v_sb = persist.tile([P, NKT, HLOC, 72], F32R, tag="v_sb")
            # attention output, transposed: [head-dim rows, ko, seq]
            attnT = persist.tile([P, 2, S], F32R, tag="attnT")

            # ---------------- Phase B/C: projections + rope + V -------------
            with tc.tile_pool(name="bc", bufs=1) as bc, \
                 tc.tile_pool(name="bcw", bufs=3) as bcw, \
                 tc.tile_pool(name="bcp", bufs=2, space="PSUM") as bcp:
                xT_sb = bc.tile([P, KO, S], F32R, tag="xT_sb")
                xT_ap = xT[:].rearrange("(ko p) s -> p ko s", p=P)
                for ko in range(KO):
                    nc.sync.dma_start(xT_sb[:, ko, :], xT_ap[:, ko, :])

                wq_sb = bc.tile([P, KO, 256], F32R, tag="wq_sb")
                wk_sb = bc.tile([P, KO, 256], F32R, tag="wk_sb")
                wv_sb = bc.tile([P, KO, 256], F32R, tag="wv_sb")
                for t, d in ((wq_sb, wq_t), (wk_sb, wk_t), (wv_sb, wv_t)):
                    nc.sync.dma_start(
                        t[:], d[:].rearrange("(ko p) m -> p ko m", p=P)
                    )
                cos_sb = bc.tile([P, S], F32, tag="cos_sb")
                sin_sb = bc.tile([P, S], F32, tag="sin_sb")
                nc.sync.dma_start(cos_sb[:], cosT[:])
                nc.sync.dma_start(sin_sb[:], sinT[:])
                perm_sb = bc.tile([P, P], F32R, tag="perm_sb")
                nc.sync.dma_start(perm_sb[:], perm[:])

                # ones column for the denominator trick (memset can't write
                # f32r directly; bounce through an f32 tile + DVE copy-cast)
                ones_sb = bc.tile([P, 1], F32, tag="ones_sb")
                nc.vector.memset(ones_sb[:], 1.0)
                nc.vector.tensor_copy(
                    out=v_sb[:, :, :, 64:65],
                    in_=ones_sb[:, None, None, :].to_broadcast((P, NKT, HLOC, 1)),
                )

                # Q/K projection + rope
                for w_sb, dest in ((wq_sb, q_rot), (wk_sb, k_rot)):
                    for hp in range(2):
                        for qc in range(NQC):
                            a_ps = bcp.tile([P, SC], F32, tag="projA")
                            for ko in range(KO):
                                nc.tensor.matmul(
                                    a_ps[:],
                                    lhsT=w_sb[:, ko, hp * P:(hp + 1) * P],
                                    rhs=xT_sb[:, ko, qc * SC:(qc + 1) * SC],
                                    start=(ko == 0),
                                    stop=(ko == KO - 1),
                                )
                            cs = slice(qc * SC, (qc + 1) * SC)
                            t2 = bcw.tile([P, SC], F32R, tag="t2")
                            nc.vector.tensor_mul(
                                out=t2[:], in0=a_ps[:], in1=sin_sb[:, cs]
                            )
                            b_ps = bcp.tile([P, SC], F32, tag="ropeB")
                            nc.tensor.matmul(
                                b_ps[:],
                                lhsT=perm_sb[:],
                                rhs=t2[:],
                                start=True,
                                stop=True,
                            )
                            dsl = dest[:, hp, cs]
                            nc.vector.tensor_mul(
                                out=dsl, in0=a_ps[:], in1=cos_sb[:, cs]
                            )
                            nc.vector.tensor_add(out=dsl, in0=dsl, in1=b_ps[:])

                # V projection
                for st in range(NKT):
                    v_ps = bcp.tile([P, 256], F32, tag="vproj")
                    for ko in range(KO):
                        nc.tensor.matmul(
                            v_ps[:],
                            lhsT=xT_sb[:, ko, st * P:(st + 1) * P],
                            rhs=wv_sb[:, ko, :],
                            start=(ko == 0),
                            stop=(ko == KO - 1),
                        )
                    nc.vector.tensor_copy(
                        out=v_sb[:, st, :, 0:64],
                        in_=v_ps[:].rearrange("p (h d) -> p h d", d=DH),
                    )

            # ---------------- Phase D: attention ----------------------------
            with tc.tile_pool(name="dp", bufs=1) as dp, \
                 tc.tile_pool(name="dw", bufs=4) as dw, \
                 tc.tile_pool(name="dn", bufs=2) as dn, \
                 tc.tile_pool(name="dps", bufs=3, space="PSUM") as dps, \
                 tc.tile_pool(name="dpa", bufs=2, space="PSUM") as dpa, \
                 tc.tile_pool(name="ddr", bufs=2, space="DRAM") as ddr:
                masks_sb = dp.tile([P, NQC, SC], F32, tag="masks_sb")
                nc.sync.dma_start(
                    masks_sb[:], masks[:].rearrange("r p f -> p r f")
                )

                for hp in range(2):
                    for hh in range(2):
                        h = 2 * hp + hh
                        hs = slice(hh * 64, (hh + 1) * 64)
                        for qc in range(NQC):
                            cs = slice(qc * SC, (qc + 1) * SC)
                            nkt_v = 4 * qc + 4
                            at_ps = dpa.tile([65, SC], F32, tag="attn")
                            for kt in range(nkt_v):
                                s_ps = dps.tile([P, SC], F32, tag="scores")
                                nc.tensor.matmul(
                                    s_ps[:],
                                    lhsT=k_rot[hs, hp, kt * P:(kt + 1) * P],
                                    rhs=q_rot[hs, hp, cs],
                                    start=True,
                                    stop=True,
                                )
                                pt = dw.tile([P, SC], F32R, tag="probs")
                                nc.scalar.activation(
                                    out=pt[:], in_=s_ps[:], func=Exp, scale=SCALE
                                )
                                r = kt - 4 * qc
                                if r >= 0:
                                    nc.vector.tensor_mul(
                                        out=pt[:], in0=pt[:], in1=masks_sb[:, r, :]
                                    )
                                nc.tensor.matmul(
                                    at_ps[:],
                                    lhsT=v_sb[:, kt, h, 0:65],
                                    rhs=pt[:],
                                    start=(kt == 0),
                                    stop=(kt == nkt_v - 1),
                                )
                            # normalize: rows 0:64 are attn, row 64 is denom
                            rt = dn.tile([P, SC], F32, tag="recip")
                            nc.vector.reciprocal(
                                out=rt[64:65, :], in_=at_ps[64:65, :]
                            )
                            dr = ddr.tile([1, SC], F32, tag="denr")
                            nc.sync.dma_start(dr[:], rt[64:65, :])
                            rbc = dn.tile([64, SC], F32, tag="rbc")
                            nc.sync.dma_start(
                                rbc[:], dr[:].partition_broadcast(64)
                            )
                            if hh == 0:
                                nc.vector.tensor_mul(
                                    out=attnT[0:64, hp, cs],
                                    in0=at_ps[0:64, :],
                                    in1=rbc[:],
                                )
                            else:
                                tmp = dn.tile([64, SC], F32R, tag="shift")
                                nc.vector.tensor_mul(
                                    out=tmp[:], in0=at_ps[0:64, :], in1=rbc[:]
                                )
                                nc.sync.dma_start(attnT[64:128, hp, cs], tmp[:])

            # ---------------- Phase E: output projection --------------------
            with tc.tile_pool(name="ep", bufs=1) as ep, \
                 tc.tile_pool(name="est", bufs=3) as est, \
                 tc.tile_pool(name="eps", bufs=2, space="PSUM") as eps:
                wo_sb = ep.tile([P, 2, DM], F32R, tag="wo_sb")
                nc.sync.dma_start(
                    wo_sb[:], wo_t[:].rearrange("(ko p) m -> p ko m", p=P)
                )
                out_ap = outp[:].rearrange("(st p) m -> p st m", p=P)
                for st in range(NKT):
                    o_t = est.tile([P, DM], F32, tag="ostg")
                    for no in range(2):
                        o_ps = eps.tile([P, SC], F32, tag="oproj")
                        for ko in range(2):
                            nc.tensor.matmul(
                                o_ps[:],
                                lhsT=attnT[:, ko, st * P:(st + 1) * P],
                                rhs=wo_sb[:, ko, no * SC:(no + 1) * SC],
                                start=(ko == 0),
                                stop=(ko == 1),
                            )
                        nc.vector.tensor_copy(
                            out=o_t[:, no * SC:(no + 1) * SC], in_=o_ps[:]
                        )
                    nc.sync.dma_start(out_ap[:, st, :], o_t[:])
    nc.compile()
    return nc


def _host_tables(token_positions):
    pos = np.asarray(token_positions).astype(np.float64)
    freq = 1.0 / (THETA ** (2.0 * np.arange(DH // 2, dtype=np.float64) / DH))
    ang = pos[:, None] * freq[None, :]  # [S, 32]
    cos_f = np.repeat(np.cos(ang), 2, axis=1)  # [S, 64]
    sin_f = np.repeat(np.sin(ang), 2, axis=1)
    cosT = np.ascontiguousarray(
        np.concatenate([cos_f.T, cos_f.T], axis=0)
    ).astype(np.float32)  # [128, S]
    sinT = np.ascontiguousarray(
        np.concatenate([sin_f.T, sin_f.T], axis=0)
    ).astype(np.float32)

    perm = np.zeros((P, P), dtype=np.float32)
    for i in range(P // 2):
        perm[2 * i + 1, 2 * i] = -1.0
        perm[2 * i, 2 * i + 1] = 1.0

    p_idx = np.arange(P)[:, None]
    f_idx = np.arange(SC)[None, :]
    masks = np.stack(
        [(f_idx >= p_idx + P * r).astype(np.float32) for r in range(NQC)]
    )  # [4, 128, 512]
    return cosT, sinT, perm, masks


_LAST_RESULTS = None


def kernel(x, wq, wk, wv, wo, token_positions):
    global _LAST_RESULTS
    from concourse.bass_utils import run_bass_kernel_spmd

    if "nc" not in _CACHE:
        _CACHE["nc"] = _build_nc()
    nc = _CACHE["nc"]

    x = np.asarray(x, dtype=np.float32)
    wq = np.asarray(wq, dtype=np.float32)
    wk = np.asarray(wk, dtype=np.float32)
    wv = np.asarray(wv, dtype=np.float32)
    wo = np.asarray(wo, dtype=np.float32)
    cosT, sinT, perm, masks = _host_tables(token_positions)

    in_maps = []
    for b in range(B):
        xT_b = np.ascontiguousarray(x[b].T)  # [DM, S]
        for g in range(4):
            rows = slice(g * 256, (g + 1) * 256)
            in_maps.append(
                {
                    "xT": xT_b,
                    "wq_t": np.ascontiguousarray(wq[rows].T),
                    "wk_t": np.ascontiguousarray(wk[rows].T),
                    "wv_t": np.ascontiguousarray(wv[rows].T),
                    "wo_t": np.ascontiguousarray(wo[:, rows].T),
                    "cosT": cosT,
                    "sinT": sinT,
                    "perm": perm,
                    "masks": masks,
                }
            )

    res = run_bass_kernel_spmd(
        nc,
        in_maps,
        core_ids=list(range(8)),
        trace=bool(os.environ.get("BASS_TRACE")),
    )
    _LAST_RESULTS = res
    outs = res.results

    out = np.zeros((B, S, DM), dtype=np.float32)
    for b in range(B):
        for g in range(4):
            out[b] += outs[b * 4 + g]["out_partial"]
    return out


# revision 16
# speedup vs baseline: 1.5689x; 1.5689x over previous
"""Causal multi-head self-attention with RoPE on 8 Trainium2 NeuronCores.

Problem: x[2, 2048, 1024] fp32, 16 heads, d_head=64, causal, RoPE(theta=1e4).
Sharding: core = b*4 + g  (b in {0,1} batch, g in {0..3} head-group of 4 heads).
Each core computes out_partial[2048, 1024] = attn(heads of g) @ wo[:, cols_g].T;
host sums the 4 partials per batch.

Per-core kernel (matmul path in bf16, fp32 PSUM accumulation):
  B) Q/K projections into [d_head, seq] layout (2 heads per 128 partitions)
     with RoPE fused:  q_rot = A*cosT + P@(A*sinT)  (P = pair-swap sign matrix,
     applied via a single PE matmul; tables are pair-symmetric so P commutes
     with the elementwise sin multiply).
  C) V projection into [seq_tile(128) partitions, 4*64+ones] layout; the
     ones column makes the second attention matmul also produce the softmax
     denominator for free.
  D) Per (head-pair, q-chunk of 512): scores_T[k 128, q 512] = K_tile @ Q_chunk
     on PE (contraction d=64; the two heads of a pair use partition halves
     0:64/64:128 so their matmuls land in different PE row groups and run
     concurrently), exp on ACT over kt-PAIRS [128, 1024] (scale=1/8 fused),
     causal masking by 0/1-mask multiply on diagonal tiles, then
     attn_aug[65, 512] += V_aug.T @ probs_T accumulated in PSUM over k tiles.
     Normalize with reciprocal_approx_fast + DRAM-bounce partition broadcast.
  E) out_partial = attnT.T @ wo_t, tiled 128x512, accumulated over 2 k-subtiles.
"""

import os
import sys

sys.path.insert(0, "/opt/trn_rl_repo")

import ml_dtypes
import numpy as np

import concourse.bacc as bacc
import concourse.mybir as mybir
from concourse.tile import TileContext

B = 2
S = 2048
DM = 1024
H = 16
DH = 64
HLOC = 4  # heads per core
SC = 512  # q chunk size
NKT = S // 128  # 16 k tiles
NQC = S // SC  # 4 q chunks
P = 128
KO = DM // P  # 8 contraction subtiles for projections
SCALE = 1.0 / 8.0  # 1/sqrt(DH)
THETA = 10000.0

F32 = mybir.dt.float32
BF16 = mybir.dt.bfloat16

_CACHE = {}


def _build_nc():
    nc = bacc.Bacc("TRN2", enable_partition_id=False)
    Exp = mybir.ActivationFunctionType.Exp

    xT = nc.dram_tensor("xT", [DM, S], BF16, kind="ExternalInput")
    wq_t = nc.dram_tensor("wq_t", [DM, 256], BF16, kind="ExternalInput")
    wk_t = nc.dram_tensor("wk_t", [DM, 256], BF16, kind="ExternalInput")
    wv_t = nc.dram_tensor("wv_t", [DM, 256], BF16, kind="ExternalInput")
    wo_t = nc.dram_tensor("wo_t", [256, DM], BF16, kind="ExternalInput")
    cosT = nc.dram_tensor("cosT", [P, S], F32, kind="ExternalInput")
    sinT = nc.dram_tensor("sinT", [P, S], F32, kind="ExternalInput")
    perm = nc.dram_tensor("perm", [P, P], BF16, kind="ExternalInput")
    masks = nc.dram_tensor("masks", [NQC, P, SC], BF16, kind="ExternalInput")
    outp = nc.dram_tensor("out_partial", [S, DM], F32, kind="ExternalOutput")

    with TileContext(nc) as tc:
        with tc.tile_pool(name="persist", bufs=1) as persist:
            # [pair-head-dim (2*64), head-pair, seq]
            q_rot = persist.tile([P, 2, S], BF16, tag="q_rot")
            k_rot = persist.tile([P, 2, S], BF16, tag="k_rot")
            # V in [k partitions, k_tile, head, 72]: cols 0:64 = V, 64 = ones
            v_sb = persist.tile([P, NKT, HLOC, 72], BF16, tag="v_sb")
            # attention output, transposed: [head-dim rows, ko, seq]
            attnT = persist.tile([P, 2, S], BF16, tag="attnT")

            # ---------------- Phase B/C: projections + rope + V -------------
            with tc.tile_pool(name="bc", bufs=1) as bc, \
                 tc.tile_pool(name="bcw", bufs=3) as bcw, \
                 tc.tile_pool(name="bcp", bufs=2, space="PSUM") as bcp:
                xT_sb = bc.tile([P, KO, S], BF16, tag="xT_sb")
                xT_ap = xT[:].rearrange("(ko p) s -> p ko s", p=P)
                for ko in range(KO):
                    nc.sync.dma_start(xT_sb[:, ko, :], xT_ap[:, ko, :])

                wq_sb = bc.tile([P, KO, 256], BF16, tag="wq_sb")
                wk_sb = bc.tile([P, KO, 256], BF16, tag="wk_sb")
                wv_sb = bc.tile([P, KO, 256], BF16, tag="wv_sb")
                for t, d in ((wq_sb, wq_t), (wk_sb, wk_t), (wv_sb, wv_t)):
                    nc.sync.dma_start(
                        t[:], d[:].rearrange("(ko p) m -> p ko m", p=P)
                    )
                cos_sb = bc.tile([P, S], F32, tag="cos_sb")
                sin_sb = bc.tile([P, S], F32, tag="sin_sb")
                nc.sync.dma_start(cos_sb[:], cosT[:])
                nc.sync.dma_start(sin_sb[:], sinT[:])
                perm_sb = bc.tile([P, P], BF16, tag="perm_sb")
                nc.sync.dma_start(perm_sb[:], perm[:])

                # ones column for the denominator trick
                nc.vector.memset(v_sb[:, :, :, 64:65], 1.0)

                # Q/K projection + rope
                for w_sb, dest in ((wq_sb, q_rot), (wk_sb, k_rot)):
                    for hp in range(2):
                        for qc in range(NQC):
                            a_ps = bcp.tile([P, SC], F32, tag="projA")
                            for ko in range(KO):
                                nc.tensor.matmul(
                                    a_ps[:],
                                    lhsT=w_sb[:, ko, hp * P:(hp + 1) * P],
                                    rhs=xT_sb[:, ko, qc * SC:(qc + 1) * SC],
                                    start=(ko == 0),
                                    stop=(ko == KO - 1),
                                )
                            cs = slice(qc * SC, (qc + 1) * SC)
                            t2 = bcw.tile([P, SC], BF16, tag="t2")
                            nc.vector.tensor_mul(
                                out=t2[:], in0=a_ps[:], in1=sin_sb[:, cs]
                            )
                            b_ps = bcp.tile([P, SC], F32, tag="ropeB")
                            nc.tensor.matmul(
                                b_ps[:],
                                lhsT=perm_sb[:],
                                rhs=t2[:],
                                start=True,
                                stop=True,
                            )
                            dsl = dest[:, hp, cs]
                            nc.vector.tensor_mul(
                                out=dsl, in0=a_ps[:], in1=cos_sb[:, cs]
                            )
                            nc.vector.tensor_add(out=dsl, in0=dsl, in1=b_ps[:])

                # V projection
                for st in range(NKT):
                    v_ps = bcp.tile([P, 256], F32, tag="vproj")
                    for ko in range(KO):
                        nc.tensor.matmul(
                            v_ps[:],
                            lhsT=xT_sb[:, ko, st * P:(st + 1) * P],
                            rhs=wv_sb[:, ko, :],
                            start=(ko == 0),
                            stop=(ko == KO - 1),
                        )
                    nc.vector.tensor_copy(
                        out=v_sb[:, st, :, 0:64],
                        in_=v_ps[:].rearrange("p (h d) -> p h d", d=DH),
                    )

            # ---------------- Phase D: attention ----------------------------
            with tc.tile_pool(name="dp", bufs=1) as dp, \
                 tc.tile_pool(name="dw", bufs=6) as dw, \
                 tc.tile_pool(name="dn", bufs=2) as dn, \
                 tc.tile_pool(name="dps", bufs=2, space="PSUM") as dps, \
                 tc.tile_pool(name="dpa", bufs=2, space="PSUM") as dpa, \
                 tc.tile_pool(name="ddr", bufs=4, space="DRAM") as ddr:
                masks_sb = dp.tile([P, NQC, SC], BF16, tag="masks_sb")
                nc.sync.dma_start(
                    masks_sb[:], masks[:].rearrange("r p f -> p r f")
                )

                for hp in range(2):
                    for qc in range(NQC):
                        cs = slice(qc * SC, (qc + 1) * SC)
                        nkt_v = 4 * qc + 4
                        # one attn accumulator per head of the pair
                        at_ps = [
                            dpa.tile([65, SC], F32, tag=f"attn{hh}",
                                     name=f"at_ps{hh}")
                            for hh in range(2)
                        ]
                        for kp in range(nkt_v // 2):
                            # paired kt scores tiles, one per head; the two
                            # heads' matmuls use different PE row groups
                            s2 = [
                                dps.tile([P, 2, SC], F32, tag="scores",
                                         name=f"s2_{hh2}")
                                for hh2 in range(2)
                            ]
                            for j in range(2):
                                kt = 2 * kp + j
                                for hh in range(2):
                                    hs = slice(hh * 64, (hh + 1) * 64)
                                    nc.tensor.matmul(
                                        s2[hh][:, j, :],
                                        lhsT=k_rot[hs, hp, kt * P:(kt + 1) * P],
                                        rhs=q_rot[hs, hp, cs],
                                        start=True,
                                        stop=True,
                                    )
                            for hh in range(2):
                                h = 2 * hp + hh
                                pt = dw.tile([P, 2, SC], BF16, tag="probs")
                                nc.scalar.activation(
                                    out=pt[:], in_=s2[hh][:], func=Exp,
                                    scale=SCALE,
                                )
                                for j in range(2):
                                    kt = 2 * kp + j
                                    r = kt - 4 * qc
                                    if r >= 0:
                                        nc.vector.tensor_mul(
                                            out=pt[:, j, :],
                                            in0=pt[:, j, :],
                                            in1=masks_sb[:, r, :],
                                        )
                                for j in range(2):
                                    kt = 2 * kp + j
                                    nc.tensor.matmul(
                                        at_ps[hh][:],
                                        lhsT=v_sb[:, kt, h, 0:65],
                                        rhs=pt[:, j, :],
                                        start=(kt == 0),
                                        stop=(kt == nkt_v - 1),
                                    )
                        # normalize: rows 0:64 are attn, row 64 is denom
                        for hh in range(2):
                            # denominator row (psum partition 64) -> sbuf,
                            # bounce through DRAM to broadcast across
                            # partitions, then approx-reciprocal at base 0
                            rt = dn.tile([P, SC], F32, tag="recip")
                            nc.vector.tensor_copy(
                                out=rt[64:65, :], in_=at_ps[hh][64:65, :]
                            )
                            dr = ddr.tile([1, SC], F32, tag="denr")
                            nc.sync.dma_start(dr[:], rt[64:65, :])
                            den_bc = dn.tile([64, SC], F32, tag="den_bc")
                            nc.sync.dma_start(
                                den_bc[:], dr[:].partition_broadcast(64)
                            )
                            rbc = dn.tile([64, SC], F32, tag="rbc")
                            nc.vector.reciprocal_approx_fast(
                                out=rbc[:], in_=den_bc[:]
                            )
                            if hh == 0:
                                nc.vector.tensor_mul(
                                    out=attnT[0:64, hp, cs],
                                    in0=at_ps[hh][0:64, :],
                                    in1=rbc[:],
                                )
                            else:
                                tmp = dn.tile([64, SC], BF16, tag="shift")
                                nc.vector.tensor_mul(
                                    out=tmp[:], in0=at_ps[hh][0:64, :],
                                    in1=rbc[:],
                                )
                                nc.sync.dma_start(attnT[64:128, hp, cs], tmp[:])

            # ---------------- Phase E: output projection --------------------
            with tc.tile_pool(name="ep", bufs=1) as ep, \
                 tc.tile_pool(name="est", bufs=3) as est, \
                 tc.tile_pool(name="eps", bufs=2, space="PSUM") as eps:
                wo_sb = ep.tile([P, 2, DM], BF16, tag="wo_sb")
                nc.sync.dma_start(
                    wo_sb[:], wo_t[:].rearrange("(ko p) m -> p ko m", p=P)
                )
                out_ap = outp[:].rearrange("(st p) m -> p st m", p=P)
                for st in range(NKT):
                    o_t = est.tile([P, DM], F32, tag="ostg")
                    for no in range(2):
                        o_ps = eps.tile([P, SC], F32, tag="oproj")
                        for ko in range(2):
                            nc.tensor.matmul(
                                o_ps[:],
                                lhsT=attnT[:, ko, st * P:(st + 1) * P],
                                rhs=wo_sb[:, ko, no * SC:(no + 1) * SC],
                                start=(ko == 0),
                                stop=(ko == 1),
                            )
                        nc.vector.tensor_copy(
                            out=o_t[:, no * SC:(no + 1) * SC], in_=o_ps[:]
                        )
                    nc.sync.dma_start(out_ap[:, st, :], o_t[:])
    nc.compile()
    return nc


def _host_tables(token_positions):
    pos = np.asarray(token_positions).astype(np.float64)
    freq = 1.0 / (THETA ** (2.0 * np.arange(DH // 2, dtype=np.float64) / DH))
    ang = pos[:, None] * freq[None, :]  # [S, 32]
    cos_f = np.repeat(np.cos(ang), 2, axis=1)  # [S, 64]
    sin_f = np.repeat(np.sin(ang), 2, axis=1)
    cosT = np.ascontiguousarray(
        np.concatenate([cos_f.T, cos_f.T], axis=0)
    ).astype(np.float32)  # [128, S]
    sinT = np.ascontiguousarray(
        np.concatenate([sin_f.T, sin_f.T], axis=0)
    ).astype(np.float32)

    perm = np.zeros((P, P), dtype=ml_dtypes.bfloat16)
    for i in range(P // 2):
        perm[2 * i + 1, 2 * i] = -1.0
        perm[2 * i, 2 * i + 1] = 1.0

    p_idx = np.arange(P)[:, None]
    f_idx = np.arange(SC)[None, :]
    masks = np.stack(
        [
            (f_idx >= p_idx + P * r).astype(ml_dtypes.bfloat16)
            for r in range(NQC)
        ]
    )  # [4, 128, 512]
    return cosT, sinT, perm, masks


_LAST_RESULTS = None


def _bf16(a):
    return np.ascontiguousarray(a).astype(ml_dtypes.bfloat16)


def kernel(x, wq, wk, wv, wo, token_positions):
    global _LAST_RESULTS
    from concourse.bass_utils import run_bass_kernel_spmd

    if "nc" not in _CACHE:
        _CACHE["nc"] = _build_nc()
    nc = _CACHE["nc"]

    x = np.asarray(x, dtype=np.float32)
    wq = np.asarray(wq, dtype=np.float32)
    wk = np.asarray(wk, dtype=np.float32)
    wv = np.asarray(wv, dtype=np.float32)
    wo = np.asarray(wo, dtype=np.float32)
    cosT, sinT, perm, masks = _host_tables(token_positions)

    in_maps = []
    for b in range(B):
        xT_b = _bf16(x[b].T)  # [DM, S]
        for g in range(4):
            rows = slice(g * 256, (g + 1) * 256)
            in_maps.append(
                {
                    "xT": xT_b,
                    "wq_t": _bf16(wq[rows].T),
                    "wk_t": _bf16(wk[rows].T),
                    "wv_t": _bf16(wv[rows].T),
                    "wo_t": _bf16(wo[:, rows].T),
                    "cosT": cosT,
                    "sinT": sinT,
                    "perm": perm,
                    "masks": masks,
                }
            )

    res = run_bass_kernel_spmd(
        nc,
        in_maps,
        core_ids=list(range(8)),
        trace=bool(os.environ.get("BASS_TRACE")),
    )
    _LAST_RESULTS = res
    outs = res.results

    out = np.zeros((B, S, DM), dtype=np.float32)
    for b in range(B):
        for g in range(4):
            out[b] += outs[b * 4 + g]["out_partial"]
    return out
